# revision 5
# baseline (speedup 1.0000x reference)
"""Causal depthwise Conv1d (B=4, L=4096, D=4096, K=4) on 8 trn2 NeuronCores.

Strategy:
  - Shard channels D across the 8 cores (512 channels each) — depthwise conv is
    channel-independent, so no communication.
  - Host-side, lay each core's shard out as [B * (512/128), 128, L]: channels on
    the 128 SBUF partitions, full L contiguous on the free axis.  The conv's
    shifts along L then become free-dim slices.
  - Per [128, L] tile: tap3+bias on ScalarE (activation Identity, per-partition
    scale/bias), taps 2/1 as fused scalar_tensor_tensor MACs on VectorE, tap 0
    on GpSimd.  Every engine stays below the DMA time, so the kernel is
    HBM-bandwidth-bound (~64 MB per core).
"""
import sys

for _p in ("/opt/trn_rl_repo", "/root/.axon_site/_ro/trn_rl_repo"):
    if _p not in sys.path:
        sys.path.insert(0, _p)

import numpy as np

import concourse.bass as bass
import concourse.bacc as bacc
import concourse.mybir as mybir
import concourse.tile as tile
from concourse.bass_utils import run_bass_kernel_spmd

B, L, D, K = 4, 4096, 4096, 4
N_CORES = 8
P = 128
DC = D // N_CORES          # 512 channels per core
DG = DC // P               # 4 partition groups per core
NTILES = B * DG            # 16 [128, L] tiles per core

f32 = mybir.dt.float32

TRACE = False              # set True (e.g. from test.py) to profile
LAST_EXEC_NS = None

_nc_cache = {}


def _build():
    nc = bacc.Bacc(trn_type="TRN2")
    x = nc.dram_tensor("x", [NTILES * P, L], f32, kind="ExternalInput")
    # per-dgroup packed params: cols 0..3 = taps, col 4 = bias
    wb = nc.dram_tensor("wb", [DG * P, K + 1], f32, kind="ExternalInput")
    out = nc.dram_tensor("out", [NTILES * P, L], f32, kind="ExternalOutput")

    with tile.TileContext(nc) as tc:
        with (
            tc.tile_pool(name="const", bufs=1) as cpool,
            tc.tile_pool(name="io", bufs=3) as iopool,
            tc.tile_pool(name="acc", bufs=3) as apool,
            tc.tile_pool(name="tmp", bufs=3) as tpool,
        ):
            wtiles = []
            for g in range(DG):
                wt = cpool.tile([P, K + 1], f32, tag=f"wb{g}")
                nc.sync.dma_start(wt[:], wb[g * P:(g + 1) * P, :])
                wtiles.append(wt)

            for t in range(NTILES):
                w = wtiles[t % DG]
                xt = iopool.tile([P, L], f32)
                nc.sync.dma_start(xt[:], x[t * P:(t + 1) * P, :])
                y = apool.tile([P, L], f32)
                # tap 3 (no shift) + bias on ScalarE: y = w3 * x + b
                nc.scalar.activation(
                    y[:], xt[:], mybir.ActivationFunctionType.Identity,
                    bias=w[:, 4:5], scale=w[:, 3:4],
                )
                # tap 2 (shift 1), VectorE: y[:, 1:] += w2 * x[:, :-1]
                nc.vector.scalar_tensor_tensor(
                    y[:, 1:], xt[:, :L - 1], w[:, 2:3], y[:, 1:],
                    mybir.AluOpType.mult, mybir.AluOpType.add,
                )
                # tap 1 (shift 2), VectorE
                nc.vector.scalar_tensor_tensor(
                    y[:, 2:], xt[:, :L - 2], w[:, 1:2], y[:, 2:],
                    mybir.AluOpType.mult, mybir.AluOpType.add,
                )
                # tap 0 (shift 3): scale on ScalarE, accumulate on GpSimd
                # (walrus rejects scalar_tensor_tensor on Pool)
                tmp = tpool.tile([P, L], f32)
                nc.scalar.activation(
                    tmp[:, :L - 3], xt[:, :L - 3],
                    mybir.ActivationFunctionType.Identity,
                    bias=0.0, scale=w[:, 0:1],
                )
                nc.gpsimd.tensor_tensor(
                    y[:, 3:], tmp[:, :L - 3], y[:, 3:], mybir.AluOpType.add
                )
                nc.sync.dma_start(out[t * P:(t + 1) * P, :], y[:])
    nc.compile()
    return nc


def kernel(x, weight, bias):
    global LAST_EXEC_NS
    x = np.asarray(x, dtype=np.float32)
    weight = np.asarray(weight, dtype=np.float32)
    bias = np.asarray(bias, dtype=np.float32)

    if "nc" not in _nc_cache:
        _nc_cache["nc"] = _build()
    nc = _nc_cache["nc"]

    in_maps = []
    for c in range(N_CORES):
        sl = slice(c * DC, (c + 1) * DC)
        # [B, L, DC] -> [B, DC, L] -> rows: b*DC + dc  (tile t = b*DG + dg)
        xs = np.ascontiguousarray(x[:, :, sl].transpose(0, 2, 1)).reshape(
            NTILES * P, L
        )
        wbp = np.concatenate(
            [weight[sl], bias[sl][:, None]], axis=1
        ).astype(np.float32)
        in_maps.append({"x": xs, "wb": np.ascontiguousarray(wbp)})

    res = run_bass_kernel_spmd(
        nc, in_maps, core_ids=list(range(N_CORES)), trace=TRACE
    )
    LAST_EXEC_NS = res.exec_time_ns

    out = np.empty((B, L, D), dtype=np.float32)
    for c in range(N_CORES):
        oc = res.results[c]["out"].reshape(B, DC, L)
        out[:, :, c * DC:(c + 1) * DC] = oc.transpose(0, 2, 1)
    return out


# revision 6
# speedup vs baseline: 1.0358x; 1.0358x over previous
"""Causal depthwise Conv1d (B=4, L=4096, D=4096, K=4) on 8 trn2 NeuronCores.

Sharding: channels D split across the 8 cores (512 each) - depthwise conv is
channel-independent, no communication.  Host lays each core's shard out as
[B*4, 128, L]: channels on SBUF partitions, L contiguous on the free axis, so
the conv shifts become free-dim slices.  Per [128, 4096] tile, work is spread
over all five engines so each stays under the ~11.5us/tile DMA time
(HBM-bound: 64 MB per core):

Out cols per tile [128, 4096]:
  z (SBUF)  = (w3*x+b) + shift1(w2*x): ACT scales (2 halves each), POOL adds.
  y (PSUM)  = shift2(w1*x) + shift3(w0*x): four quarter-slots; q0 init by ACT,
              q1..q3 init by PE diag(w1) fp32 matmuls; w0 tap via DVE stt.
  out       = z + y (DVE tt, PSUM operand; cols 0-1 copied from z by POOL).
"""
import sys

for _p in ("/opt/trn_rl_repo", "/root/.axon_site/_ro/trn_rl_repo"):
    if _p not in sys.path:
        sys.path.insert(0, _p)

import numpy as np

import concourse.bacc as bacc
import concourse.mybir as mybir
import concourse.tile as tile
from concourse.bass_utils import run_bass_kernel_spmd

B, L, D, K = 4, 4096, 4096, 4
N_CORES = 8
P = 128
DC = D // N_CORES
DG = DC // P
NTILES = B * DG

M = 2050                       # z-chain half boundary (aligned with q1|q2 seam)
QS = [(2, 1026), (1026, 2050), (2050, 3074), (3074, L)]   # psum quarters

f32 = mybir.dt.float32

TRACE = False
LAST_EXEC_NS = None

_nc_cache = {}


def _build():
    nc = bacc.Bacc(trn_type="TRN2")
    x = nc.dram_tensor("x", [NTILES * P, L], f32, kind="ExternalInput")
    wb = nc.dram_tensor("wb", [P, DG * (K + 1 + P)], f32, kind="ExternalInput")
    out = nc.dram_tensor("out", [NTILES * P, L], f32, kind="ExternalOutput")

    Id = mybir.ActivationFunctionType.Identity
    mul, add = mybir.AluOpType.mult, mybir.AluOpType.add

    with tile.TileContext(nc) as tc:
        with (
            tc.tile_pool(name="const", bufs=1) as cpool,
            tc.tile_pool(name="io", bufs=4) as iopool,
            tc.tile_pool(name="zt", bufs=2) as zpool,
            tc.tile_pool(name="t2", bufs=3) as t2pool,
            tc.tile_pool(name="res", bufs=3) as rpool,
            tc.tile_pool(name="ps", bufs=4, space="PSUM") as pspool,
        ):
            call = cpool.tile([P, DG * (K + 1 + P)], f32, tag="call")
            nc.scalar.dma_start(call[:], wb[:, :])
            W = K + 1 + P

            def wcol(g, k):   # per-partition scalar AP for tap k of dgroup g
                return call[:, g * W + k:g * W + k + 1]

            def dmat(g):      # diag(w1) stationary for dgroup g
                return call[:, g * W + K + 1:(g + 1) * W]

            for t in range(NTILES):
                g = t % DG
                xt = iopool.tile([P, L], f32)
                if t < 2:
                    nc.sync.dma_start(
                        xt[:, :M], x[t * P:(t + 1) * P, :M]
                    )
                    nc.sync.dma_start(
                        xt[:, M:], x[t * P:(t + 1) * P, M:]
                    )
                else:
                    nc.sync.dma_start(xt[:], x[t * P:(t + 1) * P, :])

                z = zpool.tile([P, L], f32)
                t2 = t2pool.tile([P, L - 1], f32)
                # ACT: z-halves + t2-halves (+ q0 init below)
                nc.scalar.activation(
                    z[:, :M], xt[:, :M], Id, bias=wcol(g, 4), scale=wcol(g, 3)
                )
                nc.scalar.activation(
                    t2[:, :M - 1], xt[:, :M - 1], Id, bias=0.0, scale=wcol(g, 2)
                )
                nc.scalar.activation(
                    z[:, M:], xt[:, M:], Id, bias=wcol(g, 4), scale=wcol(g, 3)
                )
                nc.scalar.activation(
                    t2[:, M - 1:], xt[:, M - 1:L - 1], Id,
                    bias=0.0, scale=wcol(g, 2),
                )
                # POOL: z += shift1(t2)
                nc.gpsimd.tensor_tensor(
                    z[:, 1:M], z[:, 1:M], t2[:, :M - 1], add
                )
                nc.gpsimd.tensor_tensor(
                    z[:, M:], z[:, M:], t2[:, M - 1:], add
                )

                ot = rpool.tile([P, L], f32)
                nc.gpsimd.tensor_copy(ot[:, 0:2], z[:, 0:2])

                for qi, (a, b) in enumerate(QS):
                    n = b - a
                    y = pspool.tile([P, 1024], f32, tag="y")
                    # init with tap w1 (shift 2)
                    if qi == 0:
                        nc.scalar.activation(
                            y[:, :n], xt[:, a - 2:b - 2], Id,
                            bias=0.0, scale=wcol(g, 1),
                        )
                    else:
                        for c in range(0, n, 512):
                            m = min(512, n - c)
                            nc.tensor.matmul(
                                y[:, c:c + m], dmat(g),
                                xt[:, a - 2 + c:a - 2 + c + m],
                                start=True, stop=True,
                            )
                    # tap w0 (shift 3)
                    lo = max(a, 3)
                    nc.vector.scalar_tensor_tensor(
                        y[:, lo - a:n], xt[:, lo - 3:b - 3],
                        wcol(g, 0), y[:, lo - a:n], mul, add,
                    )
                    # combine
                    nc.vector.tensor_tensor(
                        ot[:, a:b], z[:, a:b], y[:, :n], add
                    )
                if t == NTILES - 1:
                    nc.scalar.dma_start(
                        out[t * P:(t + 1) * P, :M], ot[:, :M]
                    )
                    nc.scalar.dma_start(
                        out[t * P:(t + 1) * P, M:], ot[:, M:]
                    )
                else:
                    nc.scalar.dma_start(out[t * P:(t + 1) * P, :], ot[:])
    nc.compile()
    return nc


def kernel(x, weight, bias):
    global LAST_EXEC_NS
    x = np.asarray(x, dtype=np.float32)
    weight = np.asarray(weight, dtype=np.float32)
    bias = np.asarray(bias, dtype=np.float32)

    if "nc" not in _nc_cache:
        _nc_cache["nc"] = _build()
    nc = _nc_cache["nc"]

    in_maps = []
    for c in range(N_CORES):
        sl = slice(c * DC, (c + 1) * DC)
        xs = np.ascontiguousarray(x[:, :, sl].transpose(0, 2, 1)).reshape(
            NTILES * P, L
        )
        wbp = np.concatenate(
            [weight[sl], bias[sl][:, None]], axis=1
        ).astype(np.float32)
        dgm = np.zeros((DG * P, P), dtype=np.float32)
        w1 = weight[sl][:, 1]
        for g in range(DG):
            dgm[g * P:(g + 1) * P] = np.diag(w1[g * P:(g + 1) * P])
        packed = np.concatenate([wbp, dgm], axis=1)     # [DG*P, W]
        packed = packed.reshape(DG, P, K + 1 + P).transpose(1, 0, 2).reshape(
            P, DG * (K + 1 + P)
        )
        in_maps.append({"x": xs, "wb": np.ascontiguousarray(packed)})

    res = run_bass_kernel_spmd(
        nc, in_maps, core_ids=list(range(N_CORES)), trace=TRACE
    )
    LAST_EXEC_NS = res.exec_time_ns

    out = np.empty((B, L, D), dtype=np.float32)
    for c in range(N_CORES):
        oc = res.results[c]["out"].reshape(B, DC, L)
        out[:, :, c * DC:(c + 1) * DC] = oc.transpose(0, 2, 1)
    return out
